# revision 23
# baseline (speedup 1.0000x reference)
"""Causal self-attention (B=2, T=2048, C=1024, H=16, D=64) on 8 TRN2 NeuronCores.

Sharding: core = 4*b + g  (b in {0,1} batch, g in {0..3} head-group of 4 heads).
Each core computes, for its batch element and its 4 heads:
    qkv slice -> causal attention -> partial output projection
and returns a [2048, 1024] partial product of out = y @ w_proj. The host sums
the 4 head-group partials per batch (the TP unshard step) and adds the bias
terms that commute out exactly:
    out += b_qkv[v-part] @ w_proj + b_proj      (softmax rows sum to 1)
b_q / b_k are applied on-device (per-partition bias on the Q^T/K^T copy).

All matmul operands are fp16 (PSUM accumulation stays fp32): fp16 runs at
1 cycle/row on the PE and draws far less power than fp32r, avoiding the
HAM 50%-duty throttle that dominated the fp32r version. Dataflow per core:
  x natural -> PE transpose (f32r) -> x^T (fp16)
  Q^T,K^T  = (w_qk)^T x^T      [512, T]   (channel-major, 64-row head bands)
  V natural = x @ w_v          [T, 4*65]  (65th column per head = 1.0)
  per (tq-chunk 512, head):
    S^T tile = K_tile @ Q^T_chunk            (PSUM [128, 512], causal-skipped)
    P^T = exp(S^T / 8)                       (ACT, fp16; diag masked on gpsimd)
    y'^T [65, 512] += V'_tile.T @ P^T_tile   (row 64 = softmax denominator)
    y^T = y'[0:64] * broadcast(1/y'[64])     (approx recip + PE ones-broadcast)
  proj: out_chunk = y^T.T @ w_proj -> DMA out
"""

import numpy as np
from contextlib import ExitStack

import concourse.bass as bass
import concourse.mybir as mybir
import concourse.tile as tile
from concourse import bacc
from concourse.bass_utils import run_bass_kernel_spmd
from concourse.masks import make_identity, make_upper_triangular

F32 = mybir.dt.float32
F32R = mybir.dt.float32r
F16 = mybir.dt.float16
Exp = mybir.ActivationFunctionType.Exp
Identity = mybir.ActivationFunctionType.Identity
Copy = mybir.ActivationFunctionType.Copy
ADD = mybir.AluOpType.add
GE = mybir.AluOpType.is_ge

C = 1024
NKC = C // 128  # 8 contraction tiles over channels
HL = 4          # local heads per core
D = 64


def build_nc(T: int = 2048, enable_asserts: bool = False) -> bass.Bass:
    TT = T // 128   # T tiles
    TC = T // 512   # T chunks
    assert TT == 4 * TC

    nc = bacc.Bacc(
        "TRN2",
        target_bir_lowering=False,
        debug=False,
        enable_asserts=enable_asserts,
        num_devices=8,
    )
    x_d = nc.dram_tensor("x", [T, C], F32, kind="ExternalInput").ap()
    wqkv_d = nc.dram_tensor("wqkv", [C, 768], F32, kind="ExternalInput").ap()
    bqkv_d = nc.dram_tensor("bqkv", [768], F32, kind="ExternalInput").ap()
    wproj_d = nc.dram_tensor("wproj", [256, C], F32, kind="ExternalInput").ap()
    out_d = nc.dram_tensor("out", [T, C], F32, kind="ExternalOutput").ap()

    with tile.TileContext(nc) as tc, ExitStack() as ctx:
        const = ctx.enter_context(tc.tile_pool(name="const", bufs=1))
        main = ctx.enter_context(tc.tile_pool(name="main", bufs=1))
        pt_pool = ctx.enter_context(tc.tile_pool(name="pt", bufs=8))
        small = ctx.enter_context(tc.tile_pool(name="small", bufs=2))
        ych_pool = ctx.enter_context(tc.tile_pool(name="ych", bufs=2))
        out_pool = ctx.enter_context(tc.tile_pool(name="osb", bufs=2))

        ident16 = const.tile([128, 128], F16)
        make_identity(nc, ident16)
        ones64 = const.tile([1, 64], F32)
        ones64f = const.tile([1, 64], F32)
        nc.vector.memset(ones64f, 1.0)
        nc.vector.tensor_copy(ones64.bitcast(F32R), ones64f)
        bqk = const.tile([128, 4], F32)
        nc.sync.dma_start(bqk, bqkv_d[0:512].rearrange("(m p) -> p m", p=128))

        wqkv_sb = main.tile([128, NKC, 768], F16)
        wproj_sb = main.tile([128, 2, C], F16)

        # qkT[p, m, t] = (x @ w_qk + b_qk)^T at channel u=128m+p (u<256: Q, else K)
        qkT = main.tile([128, 4, T], F16)
        # vsb[p, tt, 65h+d] = V[128tt+p, 64h+d]; column 65h+64 = 1.0
        vsb = main.tile([128, TT, HL * 65], F16)
        v4 = vsb.rearrange("p t (h e) -> p t h e", e=65)
        nc.vector.memset(v4[:, :, :, 64:65], 1.0)
        xt = main.tile([128, NKC, T], F16)

        with (
            tc.tile_pool(name="xnat", bufs=3) as xnat_pool,
            tc.tile_pool(name="x16", bufs=3) as x16_pool,
            tc.tile_pool(name="ps_tr", bufs=2, space="PSUM") as ps_tr,
            tc.tile_pool(name="ps_v", bufs=2, space="PSUM") as ps_v,
            tc.tile_pool(name="ps_qkv", bufs=4, space="PSUM") as ps_qkv,
        ):
            # x tiles 0-1 first so the PE starts transposing immediately;
            # the 4MB of weights follows on the same queue
            xns = {}
            for i in range(2):
                xns[i] = xnat_pool.tile([128, C], F32, tag="xn", name=f"xn{i}")
                nc.sync.dma_start(xns[i], x_d[128 * i : 128 * (i + 1), :])

            # weights via fp32 staging, cast to fp16 (DVE/ACT alternating)
            wqkv_r = wqkv_d.rearrange("(ko p) n -> p ko n", p=128)
            for kc in range(NKC):
                st = xnat_pool.tile([128, C], F32, tag="wst", bufs=2, name=f"wst{kc}")
                nc.sync.dma_start(st[:, 0:768], wqkv_r[:, kc, :])
                if kc % 2 == 0:
                    nc.vector.tensor_copy(wqkv_sb[:, kc, :], st[:, 0:768])
                else:
                    nc.scalar.activation(wqkv_sb[:, kc, :], st[:, 0:768], Copy)
            wproj_r = wproj_d.rearrange("(ko p) n -> p ko n", p=128)
            for j in range(2):
                st = xnat_pool.tile([128, C], F32, tag="wst", bufs=2, name=f"wpst{j}")
                nc.sync.dma_start(st, wproj_r[:, j, :])
                if j == 0:
                    nc.vector.tensor_copy(wproj_sb[:, j, :], st)
                else:
                    nc.scalar.activation(wproj_sb[:, j, :], st, Copy)

            # x^T: xt[p, kc, t] = x[t, 128kc+p] via fp16 cast (ACT) + PE
            # transpose at 1 cycle/row; 4 transposes share one PSUM tile,
            # one banded fp16 copy out (DVE 2x mode)
            for i in range(TT):
                if i in xns:
                    xn = xns[i]
                else:
                    xn = xnat_pool.tile([128, C], F32, tag="xn")
                    nc.sync.dma_start(xn, x_d[128 * i : 128 * (i + 1), :])
                xn16 = x16_pool.tile([128, C], F16, tag="xn16")
                nc.scalar.activation(xn16, xn, Copy)
                for half in range(2):
                    ptr = ps_tr.tile([128, 512], F16, tag="tr")
                    for j in range(4):
                        jj = 4 * half + j
                        nc.tensor.transpose(
                            ptr[:, 128 * j : 128 * (j + 1)],
                            xn16[:, 128 * jj : 128 * (jj + 1)],
                            ident16,
                        )
                    nc.vector.tensor_copy(
                        xt[:, 4 * half : 4 * half + 4, 128 * i : 128 * (i + 1)],
                        ptr.rearrange("p (a b) -> p a b", b=128),
                    )

                # V natural rows for tile i (stationary = x^T tile i)
                psv = ps_v.tile([128, 512], F32, tag="vps")
                for kc in range(NKC):
                    nc.tensor.matmul(
                        psv[:, 0:256],
                        lhsT=xt[:, kc, 128 * i : 128 * (i + 1)],
                        rhs=wqkv_sb[:, kc, 512:768],
                        start=(kc == 0),
                        stop=(kc == NKC - 1),
                    )
                nc.vector.tensor_copy(
                    v4[:, i, :, 0:64],
                    psv[:, 0:256].rearrange("p (h e) -> p h e", e=64),
                )

                # Q^T,K^T for chunk t once its 4 x^T tiles are in
                if i % 4 == 3:
                    t = i // 4
                    for m in range(4):
                        ps = ps_qkv.tile([128, 512], F32, tag="qkps")
                        for kc in range(NKC):
                            nc.tensor.matmul(
                                ps,
                                lhsT=wqkv_sb[:, kc, 128 * m : 128 * (m + 1)],
                                rhs=xt[:, kc, 512 * t : 512 * (t + 1)],
                                start=(kc == 0),
                                stop=(kc == NKC - 1),
                            )
                        nc.vector.tensor_tensor(
                            qkT[:, m, 512 * t : 512 * (t + 1)],
                            ps,
                            bqk[:, m : m + 1].to_broadcast([128, 512]),
                            ADD,
                        )

        # attention-phase PSUM pools (created after phase-0/1 pools are freed)
        ps_s = ctx.enter_context(tc.tile_pool(name="ps_s", bufs=2, space="PSUM"))
        ps_y = ctx.enter_context(tc.tile_pool(name="ps_y", bufs=3, space="PSUM"))
        ps_bc = ctx.enter_context(tc.tile_pool(name="ps_bc", bufs=1, space="PSUM"))
        ps_pr = ctx.enter_context(tc.tile_pool(name="ps_pr", bufs=2, space="PSUM"))

        # attention + projection, chunk-major so proj/out-DMA overlap later
        # chunks; heads processed in pairs with interleaved S / AV matmuls
        # (pair members sit in complementary PE quadrants, po 0/64) so the
        # PE never waits on the tail exp/mask of a single head.
        for c in range(TC):
            yT = ych_pool.tile([128, 2, 512], F16, tag="yT")
            ntk = 4 * c + 4
            for hp in range(HL // 2):
                pts = {0: [], 1: []}
                for tk in range(ntk):
                    jd = tk - 4 * c  # >=0 on diagonal tiles
                    off = 128 * jd if jd > 0 else 0
                    for sub in range(2):
                        h = 2 * hp + sub
                        po = 64 * (h % 2)
                        mq = h // 2
                        mk = 2 + h // 2
                        sp = ps_s.tile([128, 512], F32, tag="sp")
                        nc.tensor.matmul(
                            sp[:, off:512],
                            lhsT=qkT[po : po + 64, mk, 128 * tk : 128 * (tk + 1)],
                            rhs=qkT[po : po + 64, mq, 512 * c + off : 512 * (c + 1)],
                            start=True,
                            stop=True,
                        )
                        pt = pt_pool.tile([128, 512], F16, tag="pt")
                        nc.scalar.activation(
                            pt[:, off:512], sp[:, off:512], Exp, scale=0.125
                        )
                        if jd >= 0:
                            # causal mask inside the diagonal 128x128 block:
                            # keep pt[p, q] where q >= p, else 0 (on gpsimd,
                            # which is otherwise idle)
                            nc.gpsimd.affine_select(
                                out=pt[:, 128 * jd : 128 * (jd + 1)],
                                in_=pt[:, 128 * jd : 128 * (jd + 1)],
                                compare_op=GE,
                                fill=0.0,
                                base=0,
                                pattern=[[1, 128]],
                                channel_multiplier=-1,
                            )
                        pts[sub].append((pt, off))

                yps = {}
                for sub in range(2):
                    yps[sub] = ps_y.tile([128, 512], F32, tag="yp", name=f"yp{sub}")
                for tk in range(ntk):
                    for sub in range(2):
                        h = 2 * hp + sub
                        pt, off = pts[sub][tk]
                        nc.tensor.matmul(
                            yps[sub][0:65, off:512],
                            lhsT=vsb[:, tk, 65 * h : 65 * (h + 1)],
                            rhs=pt[:, off:512],
                            start=(tk == 0),
                            stop=(tk == ntk - 1),
                        )

                for sub in range(2):
                    h = 2 * hp + sub
                    po = 64 * (h % 2)
                    mq = h // 2
                    yp = yps[sub]
                    # 1/denominator: approx reciprocal needs SBUF fp32 in/out
                    den = small.tile([1, 512], F32, tag="den")
                    nc.vector.tensor_copy(den, yp[64:65, 0:512])
                    rraw = small.tile([1, 512], F32, tag="rraw")
                    nc.vector.reciprocal_approx_fast(out=rraw, in_=den)
                    rden = small.tile([1, 512], F32, tag="rden")
                    nc.vector.tensor_copy(rden.bitcast(F32R), rraw)
                    bp = ps_bc.tile([64, 512], F32, tag="bp")
                    nc.tensor.matmul(
                        bp, lhsT=ones64.bitcast(F32R), rhs=rden.bitcast(F32R),
                        start=True, stop=True,
                    )
                    bc = small.tile([64, 512], F32, tag="bc")
                    nc.scalar.activation(bc, bp, Copy)
                    nc.vector.tensor_mul(yT[po : po + 64, mq, :], yp[0:64, 0:512], bc)

            for tl in range(4):
                r0 = 512 * c + 128 * tl
                for nn in range(2):
                    pp = ps_pr.tile([128, 512], F32, tag="pp")
                    for j in range(2):
                        nc.tensor.matmul(
                            pp,
                            lhsT=yT[:, j, 128 * tl : 128 * (tl + 1)],
                            rhs=wproj_sb[:, j, 512 * nn : 512 * (nn + 1)],
                            start=(j == 0),
                            stop=(j == 1),
                        )
                    osb = out_pool.tile([128, 512], F32, tag="osb", bufs=3)
                    nc.vector.tensor_copy(osb, pp)
                    nc.sync.dma_start(
                        out_d[r0 : r0 + 128, 512 * nn : 512 * (nn + 1)], osb
                    )

    nc.finalize()  # runs Bacc register allocation; walrus rejects unfinalized BIR
    return nc


_NC_CACHE = {}


def _get_nc(T: int = 2048) -> bass.Bass:
    if T not in _NC_CACHE:
        _NC_CACHE[T] = build_nc(T)
    return _NC_CACHE[T]


def make_in_maps(x, w_qkv, b_qkv, w_proj):
    """Shard full inputs into 8 per-core input maps (core = 4*b + g)."""
    x = np.ascontiguousarray(np.asarray(x, dtype=np.float32))
    w_qkv = np.asarray(w_qkv, dtype=np.float32)
    b_qkv = np.asarray(b_qkv, dtype=np.float32)
    w_proj = np.asarray(w_proj, dtype=np.float32)
    in_maps = []
    for core in range(8):
        b, g = core // 4, core % 4
        gs = slice(256 * g, 256 * (g + 1))
        wl = np.ascontiguousarray(
            np.concatenate(
                [w_qkv[:, gs], w_qkv[:, 1024:2048][:, gs], w_qkv[:, 2048:3072][:, gs]],
                axis=1,
            )
        )
        bl = np.ascontiguousarray(
            np.concatenate([b_qkv[0:1024][gs], b_qkv[1024:2048][gs], b_qkv[2048:3072][gs]])
        )
        wp = np.ascontiguousarray(w_proj[gs, :])
        in_maps.append(
            {"x": np.ascontiguousarray(x[b]), "wqkv": wl, "bqkv": bl, "wproj": wp}
        )
    return in_maps


def combine_outputs(results, b_qkv, w_proj, b_proj):
    """Unshard: sum the 4 head-group partials per batch, add commuted biases."""
    b_qkv = np.asarray(b_qkv, dtype=np.float32)
    w_proj = np.asarray(w_proj, dtype=np.float32)
    b_proj = np.asarray(b_proj, dtype=np.float32)
    outs = [r["out"] for r in results]
    out = np.stack(
        [
            outs[0] + outs[1] + outs[2] + outs[3],
            outs[4] + outs[5] + outs[6] + outs[7],
        ]
    ).astype(np.float32)
    out += (b_qkv[2048:3072] @ w_proj + b_proj)[None, None, :]
    return out


def kernel(x, w_qkv, b_qkv, w_proj, b_proj):
    in_maps = make_in_maps(x, w_qkv, b_qkv, w_proj)
    res = run_bass_kernel_spmd(_get_nc(2048), in_maps, core_ids=list(range(8)))
    return combine_outputs(res.results, b_qkv, w_proj, b_proj)


def run_traced(x, w_qkv, b_qkv, w_proj, b_proj, trace_cores=None):
    """Like kernel(), but returns (output, BassKernelResults) with an NTFF trace."""
    in_maps = make_in_maps(x, w_qkv, b_qkv, w_proj)
    res = run_bass_kernel_spmd(
        _get_nc(2048),
        in_maps,
        core_ids=list(range(8)),
        trace=True,
        trace_cores=trace_cores if trace_cores is not None else [0],
    )
    return combine_outputs(res.results, b_qkv, w_proj, b_proj), res


# revision 27
# speedup vs baseline: 1.0464x; 1.0464x over previous
"""Causal self-attention (B=2, T=2048, C=1024, H=16, D=64) on 8 TRN2 NeuronCores.

Sharding: core = 4*b + g  (b in {0,1} batch, g in {0..3} head-group of 4 heads).
Each core computes, for its batch element and its 4 heads:
    qkv slice -> causal attention -> partial output projection
and returns a [2048, 1024] partial product of out = y @ w_proj. The host sums
the 4 head-group partials per batch (the TP unshard step) and adds the bias
terms that commute out exactly:
    out += b_qkv[v-part] @ w_proj + b_proj      (softmax rows sum to 1)
b_q / b_k are applied on-device (per-partition bias on the Q^T/K^T copy).

All matmul operands are fp16 (PSUM accumulation stays fp32): fp16 runs at
1 cycle/row on the PE and draws far less power than fp32r, avoiding the
HAM 50%-duty throttle that dominated the fp32r version. Dataflow per core:
  x natural -> PE transpose (f32r) -> x^T (fp16)
  Q^T,K^T  = (w_qk)^T x^T      [512, T]   (channel-major, 64-row head bands)
  V natural = x @ w_v          [T, 4*65]  (65th column per head = 1.0)
  per (tq-chunk 512, head):
    S^T tile = K_tile @ Q^T_chunk            (PSUM [128, 512], causal-skipped)
    P^T = exp(S^T / 8)                       (ACT, fp16; diag masked on gpsimd)
    y'^T [65, 512] += V'_tile.T @ P^T_tile   (row 64 = softmax denominator)
    y^T = y'[0:64] * broadcast(1/y'[64])     (approx recip + PE ones-broadcast)
  proj: out_chunk = y^T.T @ w_proj -> DMA out
"""

import numpy as np
from contextlib import ExitStack

import concourse.bass as bass
import concourse.mybir as mybir
import concourse.tile as tile
from concourse import bacc
from concourse.bass_utils import run_bass_kernel_spmd
from concourse.masks import make_identity, make_upper_triangular

F32 = mybir.dt.float32
F32R = mybir.dt.float32r
F16 = mybir.dt.float16
Exp = mybir.ActivationFunctionType.Exp
Identity = mybir.ActivationFunctionType.Identity
Copy = mybir.ActivationFunctionType.Copy
ADD = mybir.AluOpType.add
GE = mybir.AluOpType.is_ge

C = 1024
NKC = C // 128  # 8 contraction tiles over channels
HL = 4          # local heads per core
D = 64


def build_nc(T: int = 2048, enable_asserts: bool = False) -> bass.Bass:
    TT = T // 128   # T tiles
    TC = T // 512   # T chunks
    assert TT == 4 * TC

    nc = bacc.Bacc(
        "TRN2",
        target_bir_lowering=False,
        debug=False,
        enable_asserts=enable_asserts,
        num_devices=8,
    )
    x_d = nc.dram_tensor("x", [T, C], F32, kind="ExternalInput").ap()
    wqkv_d = nc.dram_tensor("wqkv", [C, 768], F32, kind="ExternalInput").ap()
    bqkv_d = nc.dram_tensor("bqkv", [768], F32, kind="ExternalInput").ap()
    wproj_d = nc.dram_tensor("wproj", [256, C], F32, kind="ExternalInput").ap()
    out_d = nc.dram_tensor("out", [T, C], F32, kind="ExternalOutput").ap()

    with tile.TileContext(nc) as tc, ExitStack() as ctx:
        const = ctx.enter_context(tc.tile_pool(name="const", bufs=1))
        main = ctx.enter_context(tc.tile_pool(name="main", bufs=1))
        pt_pool = ctx.enter_context(tc.tile_pool(name="pt", bufs=8))
        small = ctx.enter_context(tc.tile_pool(name="small", bufs=2))
        ych_pool = ctx.enter_context(tc.tile_pool(name="ych", bufs=2))
        out_pool = ctx.enter_context(tc.tile_pool(name="osb", bufs=2))

        ident = const.tile([128, 128], F32)
        make_identity(nc, ident)
        ones64 = const.tile([1, 64], F32)
        ones64f = const.tile([1, 64], F32)
        nc.vector.memset(ones64f, 1.0)
        nc.vector.tensor_copy(ones64.bitcast(F32R), ones64f)
        bqk = const.tile([128, 4], F32)
        nc.sync.dma_start(bqk, bqkv_d[0:512].rearrange("(m p) -> p m", p=128))

        wqkv_sb = main.tile([128, NKC, 768], F16)
        wproj_sb = main.tile([128, 2, C], F16)

        # qkT[p, m, t] = (x @ w_qk + b_qk)^T at channel u=128m+p (u<256: Q, else K)
        qkT = main.tile([128, 4, T], F16)
        # vsb[p, tt, 65h+d] = V[128tt+p, 64h+d]; column 65h+64 = 1.0
        vsb = main.tile([128, TT, HL * 65], F16)
        v4 = vsb.rearrange("p t (h e) -> p t h e", e=65)
        nc.vector.memset(v4[:, :, :, 64:65], 1.0)
        xt = main.tile([128, NKC, T], F16)

        with (
            tc.tile_pool(name="xnat", bufs=3) as xnat_pool,
            tc.tile_pool(name="ps_tr", bufs=2, space="PSUM") as ps_tr,
            tc.tile_pool(name="ps_v", bufs=2, space="PSUM") as ps_v,
            tc.tile_pool(name="ps_qkv", bufs=4, space="PSUM") as ps_qkv,
        ):
            # x tiles 0-1 first so the PE starts transposing immediately;
            # the 4MB of weights follows on the same queue
            xns = {}
            for i in range(2):
                xns[i] = xnat_pool.tile([128, C], F32, tag="xn", name=f"xn{i}")
                nc.sync.dma_start(xns[i], x_d[128 * i : 128 * (i + 1), :])

            # weights via fp32 staging, cast to fp16 (DVE/ACT alternating)
            wqkv_r = wqkv_d.rearrange("(ko p) n -> p ko n", p=128)
            for kc in range(NKC):
                st = xnat_pool.tile([128, C], F32, tag="wst", bufs=2, name=f"wst{kc}")
                nc.sync.dma_start(st[:, 0:768], wqkv_r[:, kc, :])
                if kc % 2 == 0:
                    nc.vector.tensor_copy(wqkv_sb[:, kc, :], st[:, 0:768])
                else:
                    nc.scalar.activation(wqkv_sb[:, kc, :], st[:, 0:768], Copy)
            wproj_r = wproj_d.rearrange("(ko p) n -> p ko n", p=128)
            for j in range(2):
                st = xnat_pool.tile([128, C], F32, tag="wst", bufs=2, name=f"wpst{j}")
                nc.sync.dma_start(st, wproj_r[:, j, :])
                if j == 0:
                    nc.vector.tensor_copy(wproj_sb[:, j, :], st)
                else:
                    nc.scalar.activation(wproj_sb[:, j, :], st, Copy)

            # x^T: xt[p, kc, t] = x[t, 128kc+p]; 4 transposes share one PSUM
            # tile, one banded copy-cast out (DVE/ACT alternating)
            for i in range(TT):
                if i in xns:
                    xn = xns[i]
                else:
                    xn = xnat_pool.tile([128, C], F32, tag="xn")
                    nc.sync.dma_start(xn, x_d[128 * i : 128 * (i + 1), :])
                for half in range(2):
                    ptr = ps_tr.tile([128, 512], F32, tag="tr")
                    for j in range(4):
                        jj = 4 * half + j
                        nc.tensor.transpose(
                            ptr[:, 128 * j : 128 * (j + 1)],
                            xn[:, 128 * jj : 128 * (jj + 1)],
                            ident,
                        )
                    dst = xt[:, 4 * half : 4 * half + 4, 128 * i : 128 * (i + 1)]
                    src = ptr.rearrange("p (a b) -> p a b", b=128)
                    if i % 2 == 0:
                        nc.vector.tensor_copy(dst, src)
                    else:
                        nc.scalar.activation(dst, src, Copy)

                # V natural rows for tile i (stationary = x^T tile i)
                psv = ps_v.tile([128, 512], F32, tag="vps")
                for kc in range(NKC):
                    nc.tensor.matmul(
                        psv[:, 0:256],
                        lhsT=xt[:, kc, 128 * i : 128 * (i + 1)],
                        rhs=wqkv_sb[:, kc, 512:768],
                        start=(kc == 0),
                        stop=(kc == NKC - 1),
                    )
                nc.vector.tensor_copy(
                    v4[:, i, :, 0:64],
                    psv[:, 0:256].rearrange("p (h e) -> p h e", e=64),
                )

                # Q^T,K^T for chunk t once its 4 x^T tiles are in
                if i % 4 == 3:
                    t = i // 4
                    for m in range(4):
                        ps = ps_qkv.tile([128, 512], F32, tag="qkps")
                        for kc in range(NKC):
                            nc.tensor.matmul(
                                ps,
                                lhsT=wqkv_sb[:, kc, 128 * m : 128 * (m + 1)],
                                rhs=xt[:, kc, 512 * t : 512 * (t + 1)],
                                start=(kc == 0),
                                stop=(kc == NKC - 1),
                            )
                        nc.vector.tensor_tensor(
                            qkT[:, m, 512 * t : 512 * (t + 1)],
                            ps,
                            bqk[:, m : m + 1].to_broadcast([128, 512]),
                            ADD,
                        )

        # attention-phase PSUM pools (created after phase-0/1 pools are freed)
        ps_s = ctx.enter_context(tc.tile_pool(name="ps_s", bufs=2, space="PSUM"))
        ps_y = ctx.enter_context(tc.tile_pool(name="ps_y", bufs=3, space="PSUM"))
        ps_bc = ctx.enter_context(tc.tile_pool(name="ps_bc", bufs=1, space="PSUM"))
        ps_pr = ctx.enter_context(tc.tile_pool(name="ps_pr", bufs=2, space="PSUM"))

        # attention + projection, chunk-major so proj/out-DMA overlap later
        # chunks; heads processed in pairs with interleaved S / AV matmuls
        # (pair members sit in complementary PE quadrants, po 0/64) so the
        # PE never waits on the tail exp/mask of a single head.
        for c in range(TC):
            yT = ych_pool.tile([128, 2, 512], F16, tag="yT")
            ntk = 4 * c + 4
            for hp in range(HL // 2):
                pts = {0: [], 1: []}
                for tk in range(ntk):
                    jd = tk - 4 * c  # >=0 on diagonal tiles
                    off = 128 * jd if jd > 0 else 0
                    for sub in range(2):
                        h = 2 * hp + sub
                        po = 64 * (h % 2)
                        mq = h // 2
                        mk = 2 + h // 2
                        sp = ps_s.tile([128, 512], F32, tag="sp")
                        nc.tensor.matmul(
                            sp[:, off:512],
                            lhsT=qkT[po : po + 64, mk, 128 * tk : 128 * (tk + 1)],
                            rhs=qkT[po : po + 64, mq, 512 * c + off : 512 * (c + 1)],
                            start=True,
                            stop=True,
                        )
                        pt = pt_pool.tile([128, 512], F16, tag="pt")
                        nc.scalar.activation(
                            pt[:, off:512], sp[:, off:512], Exp, scale=0.125
                        )
                        if jd >= 0:
                            # causal mask inside the diagonal 128x128 block:
                            # keep pt[p, q] where q >= p, else 0 (on gpsimd,
                            # which is otherwise idle)
                            nc.gpsimd.affine_select(
                                out=pt[:, 128 * jd : 128 * (jd + 1)],
                                in_=pt[:, 128 * jd : 128 * (jd + 1)],
                                compare_op=GE,
                                fill=0.0,
                                base=0,
                                pattern=[[1, 128]],
                                channel_multiplier=-1,
                            )
                        pts[sub].append((pt, off))

                yps = {}
                for sub in range(2):
                    yps[sub] = ps_y.tile([128, 512], F32, tag="yp", name=f"yp{sub}")
                for tk in range(ntk):
                    for sub in range(2):
                        h = 2 * hp + sub
                        pt, off = pts[sub][tk]
                        nc.tensor.matmul(
                            yps[sub][0:65, off:512],
                            lhsT=vsb[:, tk, 65 * h : 65 * (h + 1)],
                            rhs=pt[:, off:512],
                            start=(tk == 0),
                            stop=(tk == ntk - 1),
                        )

                for sub in range(2):
                    h = 2 * hp + sub
                    po = 64 * (h % 2)
                    mq = h // 2
                    yp = yps[sub]
                    # 1/denominator: approx reciprocal needs SBUF fp32 in/out
                    den = small.tile([1, 512], F32, tag="den")
                    nc.vector.tensor_copy(den, yp[64:65, 0:512])
                    rraw = small.tile([1, 512], F32, tag="rraw")
                    nc.vector.reciprocal_approx_fast(out=rraw, in_=den)
                    rden = small.tile([1, 512], F32, tag="rden")
                    nc.vector.tensor_copy(rden.bitcast(F32R), rraw)
                    bp = ps_bc.tile([64, 512], F32, tag="bp")
                    nc.tensor.matmul(
                        bp, lhsT=ones64.bitcast(F32R), rhs=rden.bitcast(F32R),
                        start=True, stop=True,
                    )
                    bc = small.tile([64, 512], F32, tag="bc")
                    nc.vector.tensor_copy(bc, bp)
                    nc.vector.tensor_mul(yT[po : po + 64, mq, :], yp[0:64, 0:512], bc)

            for tl in range(4):
                r0 = 512 * c + 128 * tl
                for nn in range(2):
                    pp = ps_pr.tile([128, 512], F32, tag="pp")
                    for j in range(2):
                        nc.tensor.matmul(
                            pp,
                            lhsT=yT[:, j, 128 * tl : 128 * (tl + 1)],
                            rhs=wproj_sb[:, j, 512 * nn : 512 * (nn + 1)],
                            start=(j == 0),
                            stop=(j == 1),
                        )
                    osb = out_pool.tile([128, 512], F32, tag="osb", bufs=3)
                    nc.vector.tensor_copy(osb, pp)
                    nc.sync.dma_start(
                        out_d[r0 : r0 + 128, 512 * nn : 512 * (nn + 1)], osb
                    )

    nc.finalize()  # runs Bacc register allocation; walrus rejects unfinalized BIR
    return nc


_NC_CACHE = {}


def _get_nc(T: int = 2048) -> bass.Bass:
    if T not in _NC_CACHE:
        _NC_CACHE[T] = build_nc(T)
    return _NC_CACHE[T]


def make_in_maps(x, w_qkv, b_qkv, w_proj):
    """Shard full inputs into 8 per-core input maps (core = 4*b + g)."""
    x = np.ascontiguousarray(np.asarray(x, dtype=np.float32))
    w_qkv = np.asarray(w_qkv, dtype=np.float32)
    b_qkv = np.asarray(b_qkv, dtype=np.float32)
    w_proj = np.asarray(w_proj, dtype=np.float32)
    in_maps = []
    for core in range(8):
        b, g = core // 4, core % 4
        gs = slice(256 * g, 256 * (g + 1))
        wl = np.ascontiguousarray(
            np.concatenate(
                [w_qkv[:, gs], w_qkv[:, 1024:2048][:, gs], w_qkv[:, 2048:3072][:, gs]],
                axis=1,
            )
        )
        bl = np.ascontiguousarray(
            np.concatenate([b_qkv[0:1024][gs], b_qkv[1024:2048][gs], b_qkv[2048:3072][gs]])
        )
        wp = np.ascontiguousarray(w_proj[gs, :])
        in_maps.append(
            {"x": np.ascontiguousarray(x[b]), "wqkv": wl, "bqkv": bl, "wproj": wp}
        )
    return in_maps


def combine_outputs(results, b_qkv, w_proj, b_proj):
    """Unshard: sum the 4 head-group partials per batch, add commuted biases."""
    b_qkv = np.asarray(b_qkv, dtype=np.float32)
    w_proj = np.asarray(w_proj, dtype=np.float32)
    b_proj = np.asarray(b_proj, dtype=np.float32)
    outs = [r["out"] for r in results]
    out = np.stack(
        [
            outs[0] + outs[1] + outs[2] + outs[3],
            outs[4] + outs[5] + outs[6] + outs[7],
        ]
    ).astype(np.float32)
    out += (b_qkv[2048:3072] @ w_proj + b_proj)[None, None, :]
    return out


def kernel(x, w_qkv, b_qkv, w_proj, b_proj):
    in_maps = make_in_maps(x, w_qkv, b_qkv, w_proj)
    res = run_bass_kernel_spmd(_get_nc(2048), in_maps, core_ids=list(range(8)))
    return combine_outputs(res.results, b_qkv, w_proj, b_proj)


def run_traced(x, w_qkv, b_qkv, w_proj, b_proj, trace_cores=None):
    """Like kernel(), but returns (output, BassKernelResults) with an NTFF trace."""
    in_maps = make_in_maps(x, w_qkv, b_qkv, w_proj)
    res = run_bass_kernel_spmd(
        _get_nc(2048),
        in_maps,
        core_ids=list(range(8)),
        trace=True,
        trace_cores=trace_cores if trace_cores is not None else [0],
    )
    return combine_outputs(res.results, b_qkv, w_proj, b_proj), res
